# revision 10
# baseline (speedup 1.0000x reference)
"""Causal self-attention (RoPE) Trainium2 kernel, 8-way sharded.

Sharding: core = (batch b in 0..1) x (head group g in 0..3, 4 heads each).
Each core computes its batch's attention for its 4 heads plus the partial
output projection; the host sums the 4 partials per batch.

Layout strategy (per core):
- host passes xT = x[b].T (fp16) so the embed dim lands on SBUF partitions.
- W_qkv columns are permuted so q^T/k^T emerge from the projection matmul
  already transposed, with RoPE even/odd dim pairs de-interleaved into
  x1/x2 partition blocks (scores are invariant to a head-dim permutation).
- all matmul operands are fp16 (1 cycle/row on PE vs 4 for fp32); PSUM
  accumulation stays fp32. End-to-end error ~5e-4.
- scores are computed transposed (sT[j,i]); softmax needs no max pass
  (|scores| < ~4) and the denominator is obtained by appending a ones
  column to V (M=65 PV matmuls). Normalization happens per i-block via a
  selector-matmul broadcast of 1/Z (DVE reciprocal_approx_fast).
- causal masking: only j<=i column ranges are computed; the single
  diagonal 128x128 block per j-tile gets a triangle multiply (DVE, fp16).

Scheduling strategy (the perf-critical part):
- Everything is one interleaved stream: per 512-seq chunk c we emit
  qk-projection+rope for chunk c, then the output projection of i-block
  c-1, then attention+normalization of i-block c. ACT does only softmax
  exp (~68us); all other elementwise work is spread over DVE and Pool so
  exp overlaps projection and PE (the real bottleneck, ~115us of moving
  rows) never waits long enough to drop out of its 2.4GHz p-state.
- PSUM is exactly 8 banks: pa+pb (proj, 1 each) + scores ring (2x2) +
  2 ctx accumulators; out-proj tiles reuse pa/pb slots (same shape) and
  the Z-broadcast tile reuses the scores ring.
"""
import sys

sys.path.insert(0, "/opt/trn_rl_repo")

import numpy as np

NUM_HEADS = 16
HEAD_DIM = 64
B, S, E = 2, 2048, 1024
HG = 4                      # heads per core
NG = NUM_HEADS // HG        # head groups
N_CORES = B * NG
F_QK = 2 * HG * HEAD_DIM    # 512 projected q+k rows per core
F_V = HG * HEAD_DIM         # 256 v cols per core
ESUB = E // 128             # 8 K-subtiles over embed dim
NCHUNK = 4                  # 512-col seq chunks (projection)
CHUNK = S // NCHUNK         # 512
NST = S // 128              # 16 seq tiles of 128
BLK = 512                   # attention i-block width
NBLK = S // BLK             # 4

_CACHE = {}


def _build_program():
    import concourse.bass as bass
    import concourse.mybir as mybir
    import concourse.tile as tile
    from concourse import bacc

    f32 = mybir.dt.float32
    f16 = mybir.dt.float16
    Alu = mybir.AluOpType
    Act = mybir.ActivationFunctionType

    nc = bacc.Bacc("TRN2", target_bir_lowering=False, debug=False,
                   num_devices=N_CORES)

    xT_d = nc.dram_tensor("xT", (E, S), f16, kind="ExternalInput").ap()
    wqk_d = nc.dram_tensor("wqk", (E, F_QK), f16, kind="ExternalInput").ap()
    wv_d = nc.dram_tensor("wv", (E, F_V), f16, kind="ExternalInput").ap()
    wout_d = nc.dram_tensor("wout", (F_V, E), f16, kind="ExternalInput").ap()
    cs_d = nc.dram_tensor("cs", (128, S), f32, kind="ExternalInput").ap()
    sn_d = nc.dram_tensor("sn", (128, S), f32, kind="ExternalInput").ap()
    tri_d = nc.dram_tensor("tri", (128, 128), f16, kind="ExternalInput").ap()
    sel_d = nc.dram_tensor("sel", (4, 256), f16, kind="ExternalInput").ap()
    out_d = nc.dram_tensor("out", (S, E), f16, kind="ExternalOutput").ap()

    scale = 1.0 / float(np.sqrt(HEAD_DIM))

    with tile.TileContext(nc) as tc:
        with tc.tile_pool(name="wk", bufs=1) as wp, \
             tc.tile_pool(name="rsc", bufs=3) as rsc, \
             tc.tile_pool(name="pt", bufs=4) as ptp, \
             tc.tile_pool(name="sm", bufs=2) as smp, \
             tc.tile_pool(name="ot", bufs=4) as otp, \
             tc.tile_pool(name="pp", bufs=1, space="PSUM") as pp:
            # ---- persistent SBUF tensors ----
            xT_sb = wp.tile([128, ESUB, S], f16)
            wv_sb = wp.tile([128, ESUB, F_V], f16)
            wqk_sb = wp.tile([128, ESUB, F_QK], f16)
            wout_sb = wp.tile([128, 2, E], f16)
            cs_sb = wp.tile([128, S], f32)
            sn_sb = wp.tile([128, S], f32)
            tri_sb = wp.tile([128, 128], f16)
            sel_sb = wp.tile([4, 256], f16)
            v_sb = wp.tile([128, NST, HG * 65], f16)
            ctxu_sb = wp.tile([128, 2, S], f16)
            zall32 = wp.tile([4, S], f32)
            zall16 = wp.tile([4, S], f16)
            qra = wp.tile([128, S], f16)
            qrb = wp.tile([128, S], f16)
            kra = wp.tile([128, S], f16)
            krb = wp.tile([128, S], f16)
            qp = wp.tile([128, 2, S], f16)
            kp = wp.tile([128, 2, S], f16)

            # ---- input DMAs, ordered so vproj can start ASAP ----
            xT_r = xT_d.rearrange("(o p) s -> p o s", p=128)
            nc.sync.dma_start(wv_sb[:], wv_d.rearrange("(o p) f -> p o f", p=128))
            nc.sync.dma_start(xT_sb[:, :, 0:CHUNK], xT_r[:, :, 0:CHUNK])
            nc.sync.dma_start(wqk_sb[:], wqk_d.rearrange("(o p) f -> p o f", p=128))
            nc.sync.dma_start(cs_sb[:], cs_d[:])
            nc.sync.dma_start(sn_sb[:], sn_d[:])
            for c in range(1, NCHUNK):
                csl = slice(c * CHUNK, (c + 1) * CHUNK)
                nc.sync.dma_start(xT_sb[:, :, csl], xT_r[:, :, csl])
            nc.sync.dma_start(tri_sb[:], tri_d[:])
            nc.sync.dma_start(sel_sb[:], sel_d[:])
            nc.sync.dma_start(wout_sb[:],
                              wout_d.rearrange("(o p) e -> p o e", p=128))

            # ones columns of v (only the 65th col of each head slot)
            nc.gpsimd.memset(
                v_sb[:].rearrange("p st (h w) -> p st h w", h=HG)[:, :, :, 64:65],
                1.0)

            # ---- v projection ----
            for st in range(NST):
                ssl = slice(st * 128, (st + 1) * 128)
                pv = pp.tile([128, CHUNK], f32, tag="pa", name="pv")
                for e in range(ESUB):
                    nc.tensor.matmul(pv[:, 0:F_V], xT_sb[:, e, ssl],
                                     wv_sb[:, e, :],
                                     start=(e == 0), stop=(e == ESUB - 1))
                nc.scalar.copy(
                    v_sb[:, st, :].rearrange("p (h w) -> p h w", h=HG)[:, :, 0:64],
                    pv[:, 0:F_V].rearrange("p (h w) -> p h w", h=HG))

            # ---- helpers ----
            def emit_qkproj(c):
                csl = slice(c * CHUNK, (c + 1) * CHUNK)
                for (f0, ra, rb, dst) in ((0, qra, qrb, qp),
                                          (256, kra, krb, kp)):
                    pa = pp.tile([128, CHUNK], f32, tag="pa", name="pa")
                    pb = pp.tile([128, CHUNK], f32, tag="pb", name="pb")
                    for e in range(ESUB):
                        kw = dict(start=(e == 0), stop=(e == ESUB - 1))
                        xs = xT_sb[:, e, csl]
                        nc.tensor.matmul(pa[:], wqk_sb[:, e, f0:f0 + 128], xs, **kw)
                        nc.tensor.matmul(pb[:], wqk_sb[:, e, f0 + 128:f0 + 256], xs, **kw)
                    # rope split across DVE and Pool
                    t1 = rsc.tile([128, CHUNK], f32, tag="t1", name="t1")
                    t2 = rsc.tile([128, CHUNK], f32, tag="t2", name="t2")
                    nc.vector.tensor_tensor(t1[:], pa[:], cs_sb[:, csl], Alu.mult)
                    nc.vector.tensor_tensor(t2[:], pb[:], sn_sb[:, csl], Alu.mult)
                    nc.gpsimd.tensor_tensor(ra[:, csl], t1[:], t2[:], Alu.subtract)
                    t3 = rsc.tile([128, CHUNK], f32, tag="t1", name="t3")
                    t4 = rsc.tile([128, CHUNK], f32, tag="t2", name="t4")
                    nc.vector.tensor_tensor(t3[:], pa[:], sn_sb[:, csl], Alu.mult)
                    nc.vector.tensor_tensor(t4[:], pb[:], cs_sb[:, csl], Alu.mult)
                    nc.gpsimd.tensor_tensor(rb[:, csl], t3[:], t4[:], Alu.add)
                    for p in range(2):
                        h0, h1 = 2 * p, 2 * p + 1
                        nc.sync.dma_start(dst[0:32, p, csl],
                                          ra[32 * h0:32 * h0 + 32, csl])
                        nc.sync.dma_start(dst[32:64, p, csl],
                                          rb[32 * h0:32 * h0 + 32, csl])
                        nc.sync.dma_start(dst[64:96, p, csl],
                                          ra[32 * h1:32 * h1 + 32, csl])
                        nc.sync.dma_start(dst[96:128, p, csl],
                                          rb[32 * h1:32 * h1 + 32, csl])

            def emit_attn(bb):
                # scores+exp are emitted one j-tile AHEAD of the PV matmuls
                # so exp(jt) on ACT hides under PV(jt-1)+scores(jt+1) on PE
                # instead of stalling the PE between scores(jt) and PV(jt).
                i0 = bb * BLK
                njt = 4 * bb + 4

                def s_and_e(p, jt):
                    r = jt - 4 * bb
                    off = 128 * max(r, 0)
                    ps_s = pp.tile([128, 2, BLK], f32, tag="s",
                                   name="ps_s", bufs=2)
                    for a in range(2):
                        nc.tensor.matmul(
                            ps_s[:, a, off:],
                            kp[64 * a:64 * a + 64, p,
                               128 * jt:128 * jt + 128],
                            qp[64 * a:64 * a + 64, p,
                               i0 + off:i0 + BLK],
                            start=True, stop=True)
                    pt = ptp.tile([128, 2, BLK], f16, tag="pt", name="pt")
                    nc.scalar.activation(pt[:, :, off:], ps_s[:, :, off:],
                                         Act.Exp, scale=scale)
                    if r >= 0:
                        nc.vector.tensor_tensor(
                            pt[:, :, off:off + 128],
                            pt[:, :, off:off + 128],
                            tri_sb[:, None, :].to_broadcast((128, 2, 128)),
                            Alu.mult)
                    return pt, off

                for p in range(2):
                    ctx = [pp.tile([65, BLK], f32, tag=f"ctx{a}",
                                   name=f"ctx{a}") for a in range(2)]

                    def pv(jt, pt, off):
                        # per-element has_written handles the ragged causal
                        # ranges; the 2KB-granular group check cannot
                        for a in range(2):
                            nc.tensor.matmul(
                                ctx[a][:, off:],
                                v_sb[:, jt, 65 * (2 * p + a):
                                     65 * (2 * p + a) + 65],
                                pt[:, a, off:],
                                start=(jt == 0), stop=(jt == njt - 1),
                                skip_group_check=True)

                    prev = None
                    for jt in range(njt):
                        cur = s_and_e(p, jt)
                        if prev is not None:
                            pv(jt - 1, *prev)
                        prev = cur
                    pv(njt - 1, *prev)
                    # stash unnormalized ctx + Z rows (DVE only: ACT must
                    # stay exp-only, it is the attention-phase ceiling)
                    for a in range(2):
                        nc.vector.tensor_copy(
                            ctxu_sb[64 * a:64 * a + 64, p, i0:i0 + BLK],
                            ctx[a][0:64, :])
                        zst = smp.tile([1, BLK], f32, tag="zst", name="zst",
                                       bufs=4)
                        nc.vector.tensor_copy(zst[:], ctx[a][64:65, :])
                        nc.sync.dma_start(
                            zall32[2 * p + a:2 * p + a + 1, i0:i0 + BLK],
                            zst[:])

            def emit_norm(bb):
                qsl = slice(bb * BLK, (bb + 1) * BLK)
                rz = smp.tile([4, BLK], f32, tag="rz", name="rz")
                nc.vector.reciprocal_approx_fast(rz[:], zall32[:, qsl])
                nc.vector.tensor_copy(zall16[:, qsl], rz[:])
                for p in range(2):
                    zt = pp.tile([128, 2, BLK], f32, tag="s", name="zt",
                                 bufs=2)
                    nc.tensor.matmul(zt[:, 0, :],
                                     sel_sb[:, 128 * p:128 * p + 128],
                                     zall16[:, qsl], start=True, stop=True)
                    nc.vector.tensor_tensor(ctxu_sb[:, p, qsl],
                                            ctxu_sb[:, p, qsl],
                                            zt[:, 0, :], Alu.mult)

            def emit_outproj(bb):
                # ring depth 4: rotate over pa/pb (free after projections)
                # plus the two scores slots; casts split DVE/ACT in halves
                for k in range(4):
                    st = 4 * bb + k
                    ssl = slice(st * 128, (st + 1) * 128)
                    for n in range(2):
                        i = 2 * k + n
                        if i % 4 < 2:
                            po = pp.tile([128, CHUNK], f32,
                                         tag=("pa" if i % 4 == 0 else "pb"),
                                         name="po")[:]
                        else:
                            po = pp.tile([128, 2, BLK], f32, tag="s",
                                         name="po2", bufs=2)[:, 0, :]
                        nsl = slice(n * 512, (n + 1) * 512)
                        nc.tensor.matmul(po, ctxu_sb[:, 0, ssl],
                                         wout_sb[:, 0, nsl],
                                         start=True, stop=False)
                        nc.tensor.matmul(po, ctxu_sb[:, 1, ssl],
                                         wout_sb[:, 1, nsl],
                                         start=False, stop=True)
                        ot = otp.tile([128, 512], f16, tag="ot", name="ot")
                        nc.vector.tensor_copy(ot[:, 0:256], po[:, 0:256])
                        nc.scalar.copy(ot[:, 256:512], po[:, 256:512])
                        nc.sync.dma_start(out_d[ssl, nsl], ot[:])

            # ---- interleaved main stream ----
            # outproj(bb) is emitted after qkproj(c=bb+1) so its PSUM tiles
            # (sharing pa/pb tags) slot in after the proj chains, keeping PE
            # fed while ACT works through block bb+1's exps.
            for c in range(NCHUNK):
                emit_qkproj(c)
                emit_attn(c)
                emit_norm(c)
            for bb in range(NBLK):
                emit_outproj(bb)

    nc.compile()
    return nc


def _host_inputs(x, W_qkv, W_out):
    """Build the 8 per-core input maps."""
    x = np.asarray(x, dtype=np.float32)
    W_qkv = np.asarray(W_qkv, dtype=np.float32)
    W_out = np.asarray(W_out, dtype=np.float32)

    pos = np.arange(S)
    freqs = 1.0 / 10000.0 ** (np.arange(0, HEAD_DIM, 2) / HEAD_DIM)
    ang = pos[:, None] * freqs[None, :]            # (S, 32)
    cs32 = np.cos(ang).T.astype(np.float32)        # (32, S)
    sn32 = np.sin(ang).T.astype(np.float32)
    cs = np.tile(cs32, (4, 1))                     # (128, S)
    sn = np.tile(sn32, (4, 1))
    tri = (np.arange(128)[:, None] <= np.arange(128)[None, :]).astype(np.float16)
    # selector for Z broadcast: sel[k, 128p+m] = 1 where k == 2p + m//64
    sel = np.zeros((4, 256), np.float16)
    for p in range(2):
        for m in range(128):
            sel[2 * p + m // 64, 128 * p + m] = 1.0

    in_maps = []
    for b in range(B):
        xT = np.ascontiguousarray(x[b].T.astype(np.float16))
        for g in range(NG):
            heads = np.arange(HG * g, HG * g + HG)
            qa = np.concatenate([0 * NUM_HEADS * HEAD_DIM + h * HEAD_DIM
                                 + np.arange(0, HEAD_DIM, 2) for h in heads])
            qb = qa + 1
            ka = qa + NUM_HEADS * HEAD_DIM
            kb = ka + 1
            wqk = np.ascontiguousarray(
                W_qkv[:, np.concatenate([qa, qb, ka, kb])].astype(np.float16))
            vcols = np.concatenate([2 * NUM_HEADS * HEAD_DIM + h * HEAD_DIM
                                    + np.arange(HEAD_DIM) for h in heads])
            wv = np.ascontiguousarray(W_qkv[:, vcols].astype(np.float16))
            wout = np.ascontiguousarray(
                W_out[HG * g * HEAD_DIM:HG * (g + 1) * HEAD_DIM].astype(np.float16))
            in_maps.append({"xT": xT, "wqk": wqk, "wv": wv, "wout": wout,
                            "cs": cs, "sn": sn, "tri": tri, "sel": sel})
    return in_maps


def get_program():
    if "nc" not in _CACHE:
        _CACHE["nc"] = _build_program()
    return _CACHE["nc"]


def run(x, W_qkv, W_out, trace=False, tmpdir=None):
    from concourse import bass_utils
    nc = get_program()
    in_maps = _host_inputs(x, W_qkv, W_out)
    res = bass_utils.run_bass_kernel_spmd(
        nc, in_maps, core_ids=list(range(N_CORES)), trace=trace, tmpdir=tmpdir)
    out = np.zeros((B, S, E), np.float32)
    for b in range(B):
        for g in range(NG):
            out[b] += res.results[b * NG + g]["out"].astype(np.float32)
    return out, res


def kernel(x, W_qkv, W_out):
    out, _ = run(x, W_qkv, W_out)
    return out


# revision 11
# speedup vs baseline: 1.0414x; 1.0414x over previous
"""Causal self-attention (RoPE) Trainium2 kernel, 8-way sharded.

Sharding: core = (batch b in 0..1) x (head group g in 0..3, 4 heads each).
Each core computes its batch's attention for its 4 heads plus the partial
output projection; the host sums the 4 partials per batch.

Layout strategy (per core):
- host passes xT = x[b].T (fp16) so the embed dim lands on SBUF partitions.
- W_qkv columns are permuted so q^T/k^T emerge from the projection matmul
  already transposed, with RoPE even/odd dim pairs de-interleaved into
  x1/x2 partition blocks (scores are invariant to a head-dim permutation).
- all matmul operands are fp16 (1 cycle/row on PE vs 4 for fp32); PSUM
  accumulation stays fp32. End-to-end error ~5e-4.
- scores are computed transposed (sT[j,i]); softmax needs no max pass
  (|scores| < ~4) and the denominator is obtained by appending a ones
  column to V (M=65 PV matmuls). Normalization happens per i-block via a
  selector-matmul broadcast of 1/Z (DVE reciprocal_approx_fast).
- causal masking: only j<=i column ranges are computed; the single
  diagonal 128x128 block per j-tile gets a triangle multiply (DVE, fp16).

Scheduling strategy (the perf-critical part):
- Everything is one interleaved stream: per 512-seq chunk c we emit
  qk-projection+rope for chunk c, then the output projection of i-block
  c-1, then attention+normalization of i-block c. ACT does only softmax
  exp (~68us); all other elementwise work is spread over DVE and Pool so
  exp overlaps projection and PE (the real bottleneck, ~115us of moving
  rows) never waits long enough to drop out of its 2.4GHz p-state.
- PSUM is exactly 8 banks: pa+pb (proj, 1 each) + scores ring (2x2) +
  2 ctx accumulators; out-proj tiles reuse pa/pb slots (same shape) and
  the Z-broadcast tile reuses the scores ring.
"""
import sys

sys.path.insert(0, "/opt/trn_rl_repo")

import numpy as np

NUM_HEADS = 16
HEAD_DIM = 64
B, S, E = 2, 2048, 1024
HG = 4                      # heads per core
NG = NUM_HEADS // HG        # head groups
N_CORES = B * NG
F_QK = 2 * HG * HEAD_DIM    # 512 projected q+k rows per core
F_V = HG * HEAD_DIM         # 256 v cols per core
ESUB = E // 128             # 8 K-subtiles over embed dim
NCHUNK = 4                  # 512-col seq chunks (projection)
CHUNK = S // NCHUNK         # 512
NST = S // 128              # 16 seq tiles of 128
BLK = 512                   # attention i-block width
NBLK = S // BLK             # 4

_CACHE = {}


def _build_program():
    import concourse.bass as bass
    import concourse.mybir as mybir
    import concourse.tile as tile
    from concourse import bacc

    f32 = mybir.dt.float32
    f16 = mybir.dt.float16
    Alu = mybir.AluOpType
    Act = mybir.ActivationFunctionType

    nc = bacc.Bacc("TRN2", target_bir_lowering=False, debug=False,
                   num_devices=N_CORES)

    xT_d = nc.dram_tensor("xT", (E, S), f16, kind="ExternalInput").ap()
    wqk_d = nc.dram_tensor("wqk", (E, F_QK), f16, kind="ExternalInput").ap()
    wv_d = nc.dram_tensor("wv", (E, F_V), f16, kind="ExternalInput").ap()
    wout_d = nc.dram_tensor("wout", (F_V, E), f16, kind="ExternalInput").ap()
    cs_d = nc.dram_tensor("cs", (128, S), f32, kind="ExternalInput").ap()
    sn_d = nc.dram_tensor("sn", (128, S), f32, kind="ExternalInput").ap()
    tri_d = nc.dram_tensor("tri", (128, 128), f16, kind="ExternalInput").ap()
    sel_d = nc.dram_tensor("sel", (4, 256), f16, kind="ExternalInput").ap()
    out_d = nc.dram_tensor("out", (S, E), f16, kind="ExternalOutput").ap()

    scale = 1.0 / float(np.sqrt(HEAD_DIM))

    with tile.TileContext(nc) as tc:
        with tc.tile_pool(name="wk", bufs=1) as wp, \
             tc.tile_pool(name="rsc", bufs=3) as rsc, \
             tc.tile_pool(name="pt", bufs=4) as ptp, \
             tc.tile_pool(name="sm", bufs=2) as smp, \
             tc.tile_pool(name="ot", bufs=4) as otp, \
             tc.tile_pool(name="pp", bufs=1, space="PSUM") as pp:
            # ---- persistent SBUF tensors ----
            xT_sb = wp.tile([128, ESUB, S], f16)
            wv_sb = wp.tile([128, ESUB, F_V], f16)
            wqk_sb = wp.tile([128, ESUB, F_QK], f16)
            wout_sb = wp.tile([128, 2, E], f16)
            cs_sb = wp.tile([128, S], f32)
            sn_sb = wp.tile([128, S], f32)
            tri_sb = wp.tile([128, 128], f16)
            sel_sb = wp.tile([4, 256], f16)
            v_sb = wp.tile([128, NST, HG * 65], f16)
            ctxu_sb = wp.tile([128, 2, S], f16)
            zall32 = wp.tile([4, S], f32)
            zall16 = wp.tile([4, S], f16)
            qra = wp.tile([128, S], f16)
            qrb = wp.tile([128, S], f16)
            kra = wp.tile([128, S], f16)
            krb = wp.tile([128, S], f16)
            qp = wp.tile([128, 2, S], f16)
            kp = wp.tile([128, 2, S], f16)

            # ---- input DMAs, ordered so vproj can start ASAP ----
            xT_r = xT_d.rearrange("(o p) s -> p o s", p=128)
            nc.sync.dma_start(wv_sb[:], wv_d.rearrange("(o p) f -> p o f", p=128))
            nc.sync.dma_start(xT_sb[:, :, 0:256], xT_r[:, :, 0:256])
            nc.sync.dma_start(xT_sb[:, :, 256:512], xT_r[:, :, 256:512])
            nc.sync.dma_start(wqk_sb[:], wqk_d.rearrange("(o p) f -> p o f", p=128))
            nc.sync.dma_start(cs_sb[:], cs_d[:])
            nc.sync.dma_start(sn_sb[:], sn_d[:])
            for c in range(1, NCHUNK):
                csl = slice(c * CHUNK, (c + 1) * CHUNK)
                nc.sync.dma_start(xT_sb[:, :, csl], xT_r[:, :, csl])
            nc.sync.dma_start(tri_sb[:], tri_d[:])
            nc.sync.dma_start(sel_sb[:], sel_d[:])
            nc.sync.dma_start(wout_sb[:],
                              wout_d.rearrange("(o p) e -> p o e", p=128))

            # ones columns of v (only the 65th col of each head slot)
            nc.gpsimd.memset(
                v_sb[:].rearrange("p st (h w) -> p st h w", h=HG)[:, :, :, 64:65],
                1.0)

            # ---- helpers ----
            def emit_vproj(c):
                for st in range(4 * c, 4 * c + 4):
                    ssl = slice(st * 128, (st + 1) * 128)
                    pv = pp.tile([128, CHUNK], f32, tag="pa", name="pv")
                    for e in range(ESUB):
                        nc.tensor.matmul(pv[:, 0:F_V], xT_sb[:, e, ssl],
                                         wv_sb[:, e, :],
                                         start=(e == 0), stop=(e == ESUB - 1))
                    nc.vector.tensor_copy(
                        v_sb[:, st, :].rearrange("p (h w) -> p h w", h=HG)[:, :, 0:64],
                        pv[:, 0:F_V].rearrange("p (h w) -> p h w", h=HG))

            def emit_qkproj(c):
                csl = slice(c * CHUNK, (c + 1) * CHUNK)
                for (f0, ra, rb, dst) in ((0, qra, qrb, qp),
                                          (256, kra, krb, kp)):
                    pa = pp.tile([128, CHUNK], f32, tag="pa", name="pa")
                    pb = pp.tile([128, CHUNK], f32, tag="pb", name="pb")
                    for e in range(ESUB):
                        kw = dict(start=(e == 0), stop=(e == ESUB - 1))
                        xs = xT_sb[:, e, csl]
                        nc.tensor.matmul(pa[:], wqk_sb[:, e, f0:f0 + 128], xs, **kw)
                        nc.tensor.matmul(pb[:], wqk_sb[:, e, f0 + 128:f0 + 256], xs, **kw)
                    # rope split across DVE and Pool
                    t1 = rsc.tile([128, CHUNK], f32, tag="t1", name="t1")
                    t2 = rsc.tile([128, CHUNK], f32, tag="t2", name="t2")
                    nc.vector.tensor_tensor(t1[:], pa[:], cs_sb[:, csl], Alu.mult)
                    nc.vector.tensor_tensor(t2[:], pb[:], sn_sb[:, csl], Alu.mult)
                    nc.gpsimd.tensor_tensor(ra[:, csl], t1[:], t2[:], Alu.subtract)
                    t3 = rsc.tile([128, CHUNK], f32, tag="t1", name="t3")
                    t4 = rsc.tile([128, CHUNK], f32, tag="t2", name="t4")
                    nc.vector.tensor_tensor(t3[:], pa[:], sn_sb[:, csl], Alu.mult)
                    nc.vector.tensor_tensor(t4[:], pb[:], cs_sb[:, csl], Alu.mult)
                    nc.gpsimd.tensor_tensor(rb[:, csl], t3[:], t4[:], Alu.add)
                    for p in range(2):
                        h0, h1 = 2 * p, 2 * p + 1
                        nc.sync.dma_start(dst[0:32, p, csl],
                                          ra[32 * h0:32 * h0 + 32, csl])
                        nc.sync.dma_start(dst[32:64, p, csl],
                                          rb[32 * h0:32 * h0 + 32, csl])
                        nc.sync.dma_start(dst[64:96, p, csl],
                                          ra[32 * h1:32 * h1 + 32, csl])
                        nc.sync.dma_start(dst[96:128, p, csl],
                                          rb[32 * h1:32 * h1 + 32, csl])

            def emit_attn(bb):
                # scores+exp are emitted one j-tile AHEAD of the PV matmuls
                # so exp(jt) on ACT hides under PV(jt-1)+scores(jt+1) on PE
                # instead of stalling the PE between scores(jt) and PV(jt).
                i0 = bb * BLK
                njt = 4 * bb + 4

                def s_and_e(p, jt):
                    r = jt - 4 * bb
                    off = 128 * max(r, 0)
                    ps_s = pp.tile([128, 2, BLK], f32, tag="s",
                                   name="ps_s", bufs=2)
                    for a in range(2):
                        nc.tensor.matmul(
                            ps_s[:, a, off:],
                            kp[64 * a:64 * a + 64, p,
                               128 * jt:128 * jt + 128],
                            qp[64 * a:64 * a + 64, p,
                               i0 + off:i0 + BLK],
                            start=True, stop=True)
                    pt = ptp.tile([128, 2, BLK], f16, tag="pt", name="pt")
                    nc.scalar.activation(pt[:, :, off:], ps_s[:, :, off:],
                                         Act.Exp, scale=scale)
                    if r >= 0:
                        nc.vector.tensor_tensor(
                            pt[:, :, off:off + 128],
                            pt[:, :, off:off + 128],
                            tri_sb[:, None, :].to_broadcast((128, 2, 128)),
                            Alu.mult)
                    return pt, off

                for p in range(2):
                    ctx = [pp.tile([65, BLK], f32, tag=f"ctx{a}",
                                   name=f"ctx{a}") for a in range(2)]

                    def pv(jt, pt, off):
                        # per-element has_written handles the ragged causal
                        # ranges; the 2KB-granular group check cannot
                        for a in range(2):
                            nc.tensor.matmul(
                                ctx[a][:, off:],
                                v_sb[:, jt, 65 * (2 * p + a):
                                     65 * (2 * p + a) + 65],
                                pt[:, a, off:],
                                start=(jt == 0), stop=(jt == njt - 1),
                                skip_group_check=True)

                    prev = None
                    for jt in range(njt):
                        cur = s_and_e(p, jt)
                        if prev is not None:
                            pv(jt - 1, *prev)
                        prev = cur
                    pv(njt - 1, *prev)
                    # stash unnormalized ctx + Z rows (DVE only: ACT must
                    # stay exp-only, it is the attention-phase ceiling)
                    for a in range(2):
                        nc.vector.tensor_copy(
                            ctxu_sb[64 * a:64 * a + 64, p, i0:i0 + BLK],
                            ctx[a][0:64, :])
                        zst = smp.tile([1, BLK], f32, tag="zst", name="zst",
                                       bufs=4)
                        nc.vector.tensor_copy(zst[:], ctx[a][64:65, :])
                        nc.sync.dma_start(
                            zall32[2 * p + a:2 * p + a + 1, i0:i0 + BLK],
                            zst[:])

            def emit_norm(bb):
                qsl = slice(bb * BLK, (bb + 1) * BLK)
                rz = smp.tile([4, BLK], f32, tag="rz", name="rz")
                nc.vector.reciprocal_approx_fast(rz[:], zall32[:, qsl])
                nc.vector.tensor_copy(zall16[:, qsl], rz[:])
                for p in range(2):
                    zt = pp.tile([128, 2, BLK], f32, tag="s", name="zt",
                                 bufs=2)
                    nc.tensor.matmul(zt[:, 0, :],
                                     sel_sb[:, 128 * p:128 * p + 128],
                                     zall16[:, qsl], start=True, stop=True)
                    nc.vector.tensor_tensor(ctxu_sb[:, p, qsl],
                                            ctxu_sb[:, p, qsl],
                                            zt[:, 0, :], Alu.mult)

            def emit_outproj(bb):
                # ring depth 4: rotate over pa/pb (free after projections)
                # plus the two scores slots; casts split DVE/ACT in halves
                for k in range(4):
                    st = 4 * bb + k
                    ssl = slice(st * 128, (st + 1) * 128)
                    for n in range(2):
                        i = 2 * k + n
                        if i % 4 < 2:
                            po = pp.tile([128, CHUNK], f32,
                                         tag=("pa" if i % 4 == 0 else "pb"),
                                         name="po")[:]
                        else:
                            po = pp.tile([128, 2, BLK], f32, tag="s",
                                         name="po2", bufs=2)[:, 0, :]
                        nsl = slice(n * 512, (n + 1) * 512)
                        nc.tensor.matmul(po, ctxu_sb[:, 0, ssl],
                                         wout_sb[:, 0, nsl],
                                         start=True, stop=False)
                        nc.tensor.matmul(po, ctxu_sb[:, 1, ssl],
                                         wout_sb[:, 1, nsl],
                                         start=False, stop=True)
                        ot = otp.tile([128, 512], f16, tag="ot", name="ot")
                        nc.vector.tensor_copy(ot[:, 0:256], po[:, 0:256])
                        nc.scalar.copy(ot[:, 256:512], po[:, 256:512])
                        nc.sync.dma_start(out_d[ssl, nsl], ot[:])

            # ---- main stream, software-pipelined at chunk level ----
            # projections run one full chunk ahead of the attention block
            # that consumes them, hiding the proj->rope->relayout-DMA
            # latency; outproj(bb) follows norm(bb) immediately so its
            # matmuls fill the exp-gated PE idle of the next block.
            emit_vproj(0)
            emit_qkproj(0)
            emit_vproj(1)
            emit_qkproj(1)
            for bb in range(NBLK):
                emit_attn(bb)
                emit_norm(bb)
                emit_outproj(bb)
                if bb + 2 < NCHUNK:
                    emit_vproj(bb + 2)
                    emit_qkproj(bb + 2)

    nc.compile()
    return nc


def _host_inputs(x, W_qkv, W_out):
    """Build the 8 per-core input maps."""
    x = np.asarray(x, dtype=np.float32)
    W_qkv = np.asarray(W_qkv, dtype=np.float32)
    W_out = np.asarray(W_out, dtype=np.float32)

    pos = np.arange(S)
    freqs = 1.0 / 10000.0 ** (np.arange(0, HEAD_DIM, 2) / HEAD_DIM)
    ang = pos[:, None] * freqs[None, :]            # (S, 32)
    cs32 = np.cos(ang).T.astype(np.float32)        # (32, S)
    sn32 = np.sin(ang).T.astype(np.float32)
    cs = np.tile(cs32, (4, 1))                     # (128, S)
    sn = np.tile(sn32, (4, 1))
    tri = (np.arange(128)[:, None] <= np.arange(128)[None, :]).astype(np.float16)
    # selector for Z broadcast: sel[k, 128p+m] = 1 where k == 2p + m//64
    sel = np.zeros((4, 256), np.float16)
    for p in range(2):
        for m in range(128):
            sel[2 * p + m // 64, 128 * p + m] = 1.0

    in_maps = []
    for b in range(B):
        xT = np.ascontiguousarray(x[b].T.astype(np.float16))
        for g in range(NG):
            heads = np.arange(HG * g, HG * g + HG)
            qa = np.concatenate([0 * NUM_HEADS * HEAD_DIM + h * HEAD_DIM
                                 + np.arange(0, HEAD_DIM, 2) for h in heads])
            qb = qa + 1
            ka = qa + NUM_HEADS * HEAD_DIM
            kb = ka + 1
            wqk = np.ascontiguousarray(
                W_qkv[:, np.concatenate([qa, qb, ka, kb])].astype(np.float16))
            vcols = np.concatenate([2 * NUM_HEADS * HEAD_DIM + h * HEAD_DIM
                                    + np.arange(HEAD_DIM) for h in heads])
            wv = np.ascontiguousarray(W_qkv[:, vcols].astype(np.float16))
            wout = np.ascontiguousarray(
                W_out[HG * g * HEAD_DIM:HG * (g + 1) * HEAD_DIM].astype(np.float16))
            in_maps.append({"xT": xT, "wqk": wqk, "wv": wv, "wout": wout,
                            "cs": cs, "sn": sn, "tri": tri, "sel": sel})
    return in_maps


def get_program():
    if "nc" not in _CACHE:
        _CACHE["nc"] = _build_program()
    return _CACHE["nc"]


def run(x, W_qkv, W_out, trace=False, tmpdir=None):
    from concourse import bass_utils
    nc = get_program()
    in_maps = _host_inputs(x, W_qkv, W_out)
    res = bass_utils.run_bass_kernel_spmd(
        nc, in_maps, core_ids=list(range(N_CORES)), trace=trace, tmpdir=tmpdir)
    out = np.zeros((B, S, E), np.float32)
    for b in range(B):
        for g in range(NG):
            out[b] += res.results[b * NG + g]["out"].astype(np.float32)
    return out, res


def kernel(x, W_qkv, W_out):
    out, _ = run(x, W_qkv, W_out)
    return out


# revision 14
# speedup vs baseline: 1.1390x; 1.0937x over previous
"""Causal self-attention (RoPE) Trainium2 kernel, 8-way sharded.

Sharding: core = (batch b in 0..1) x (head group g in 0..3, 4 heads each).
Each core computes its batch's attention for its 4 heads plus the partial
output projection; the host sums the 4 partials per batch.

Layout strategy (per core):
- host passes xT = x[b].T (fp16) so the embed dim lands on SBUF partitions.
- W_qkv columns are permuted so q^T/k^T emerge from the projection matmul
  already transposed, with RoPE even/odd dim pairs de-interleaved into
  x1/x2 partition blocks (scores are invariant to a head-dim permutation).
- all matmul operands are fp16 (1 cycle/row on PE vs 4 for fp32); PSUM
  accumulation stays fp32. End-to-end error ~5e-4.
- scores are computed transposed (sT[j,i]); softmax needs no max pass
  (|scores| < ~4) and the denominator is obtained by appending a ones
  column to V (M=65 PV matmuls). Normalization happens per i-block via a
  selector-matmul broadcast of 1/Z (DVE reciprocal_approx_fast).
- causal masking: only j<=i column ranges are computed; the single
  diagonal 128x128 block per j-tile gets a triangle multiply (DVE, fp16).

Scheduling strategy (the perf-critical part):
- Everything is one interleaved stream: per 512-seq chunk c we emit
  qk-projection+rope for chunk c, then the output projection of i-block
  c-1, then attention+normalization of i-block c. ACT does only softmax
  exp (~68us); all other elementwise work is spread over DVE and Pool so
  exp overlaps projection and PE (the real bottleneck, ~115us of moving
  rows) never waits long enough to drop out of its 2.4GHz p-state.
- PSUM is exactly 8 banks: pa+pb (proj, 1 each) + scores ring (2x2) +
  2 ctx accumulators; out-proj tiles reuse pa/pb slots (same shape) and
  the Z-broadcast tile reuses the scores ring.
"""
import sys

sys.path.insert(0, "/opt/trn_rl_repo")

import numpy as np

NUM_HEADS = 16
HEAD_DIM = 64
B, S, E = 2, 2048, 1024
HG = 4                      # heads per core
NG = NUM_HEADS // HG        # head groups
N_CORES = B * NG
F_QK = 2 * HG * HEAD_DIM    # 512 projected q+k rows per core
F_V = HG * HEAD_DIM         # 256 v cols per core
ESUB = E // 128             # 8 K-subtiles over embed dim
NCHUNK = 4                  # 512-col seq chunks (projection)
CHUNK = S // NCHUNK         # 512
NST = S // 128              # 16 seq tiles of 128
BLK = 512                   # attention i-block width
NBLK = S // BLK             # 4

_CACHE = {}


def _build_program():
    import concourse.bass as bass
    import concourse.mybir as mybir
    import concourse.tile as tile
    from concourse import bacc

    f32 = mybir.dt.float32
    f16 = mybir.dt.float16
    Alu = mybir.AluOpType
    Act = mybir.ActivationFunctionType

    nc = bacc.Bacc("TRN2", target_bir_lowering=False, debug=False,
                   num_devices=N_CORES)

    xT_d = nc.dram_tensor("xT", (E, S), f16, kind="ExternalInput").ap()
    wqk_d = nc.dram_tensor("wqk", (E, F_QK), f16, kind="ExternalInput").ap()
    wv_d = nc.dram_tensor("wv", (E, F_V), f16, kind="ExternalInput").ap()
    wout_d = nc.dram_tensor("wout", (F_V, E), f16, kind="ExternalInput").ap()
    cs_d = nc.dram_tensor("cs", (128, S), f32, kind="ExternalInput").ap()
    sn_d = nc.dram_tensor("sn", (128, S), f32, kind="ExternalInput").ap()
    tri_d = nc.dram_tensor("tri", (128, 128), f16, kind="ExternalInput").ap()
    sel_d = nc.dram_tensor("sel", (4, 256), f16, kind="ExternalInput").ap()
    out_d = nc.dram_tensor("out", (S, E), f16, kind="ExternalOutput").ap()

    scale = 1.0 / float(np.sqrt(HEAD_DIM))

    with tile.TileContext(nc) as tc:
        with tc.tile_pool(name="wk", bufs=1) as wp, \
             tc.tile_pool(name="rsc", bufs=3) as rsc, \
             tc.tile_pool(name="pt", bufs=4) as ptp, \
             tc.tile_pool(name="sm", bufs=2) as smp, \
             tc.tile_pool(name="ot", bufs=4) as otp, \
             tc.tile_pool(name="pp", bufs=1, space="PSUM") as pp:
            # ---- persistent SBUF tensors ----
            xT_sb = wp.tile([128, ESUB, S], f16)
            wv_sb = wp.tile([128, ESUB, F_V], f16)
            wqk_sb = wp.tile([128, ESUB, F_QK], f16)
            wout_sb = wp.tile([128, 2, E], f16)
            cs_sb = wp.tile([128, S], f32)
            sn_sb = wp.tile([128, S], f32)
            tri_sb = wp.tile([128, 128], f16)
            sel_sb = wp.tile([4, 256], f16)
            v_sb = wp.tile([128, NST, HG * 65], f16)
            ctxu_sb = wp.tile([128, 2, S], f16)
            zall32 = wp.tile([4, S], f32)
            zall16 = wp.tile([4, S], f16)
            qra = wp.tile([128, S], f16)
            qrb = wp.tile([128, S], f16)
            kra = wp.tile([128, S], f16)
            krb = wp.tile([128, S], f16)
            qp = wp.tile([128, 2, S], f16)
            kp = wp.tile([128, 2, S], f16)

            # ---- input DMAs, ordered so vproj can start ASAP ----
            xT_r = xT_d.rearrange("(o p) s -> p o s", p=128)
            nc.sync.dma_start(wv_sb[:], wv_d.rearrange("(o p) f -> p o f", p=128))
            nc.sync.dma_start(xT_sb[:, :, 0:256], xT_r[:, :, 0:256])
            nc.sync.dma_start(xT_sb[:, :, 256:512], xT_r[:, :, 256:512])
            nc.sync.dma_start(wqk_sb[:], wqk_d.rearrange("(o p) f -> p o f", p=128))
            nc.sync.dma_start(cs_sb[:], cs_d[:])
            nc.sync.dma_start(sn_sb[:], sn_d[:])
            for c in range(1, NCHUNK):
                csl = slice(c * CHUNK, (c + 1) * CHUNK)
                nc.sync.dma_start(xT_sb[:, :, csl], xT_r[:, :, csl])
            nc.sync.dma_start(tri_sb[:], tri_d[:])
            nc.sync.dma_start(sel_sb[:], sel_d[:])
            nc.sync.dma_start(wout_sb[:],
                              wout_d.rearrange("(o p) e -> p o e", p=128))

            # ones columns of v (only the 65th col of each head slot)
            nc.gpsimd.memset(
                v_sb[:].rearrange("p st (h w) -> p st h w", h=HG)[:, :, :, 64:65],
                1.0)

            # ---- emission helpers ----
            def emit_vproj(c):
                for st in range(4 * c, 4 * c + 4):
                    ssl = slice(st * 128, (st + 1) * 128)
                    pv = pp.tile([128, CHUNK], f32, tag="pa", name="pv")
                    for e in range(ESUB):
                        nc.tensor.matmul(pv[:, 0:F_V], xT_sb[:, e, ssl],
                                         wv_sb[:, e, :],
                                         start=(e == 0), stop=(e == ESUB - 1))
                    nc.vector.tensor_copy(
                        v_sb[:, st, :].rearrange("p (h w) -> p h w", h=HG)[:, :, 0:64],
                        pv[:, 0:F_V].rearrange("p (h w) -> p h w", h=HG))

            def emit_qkproj(c):
                csl = slice(c * CHUNK, (c + 1) * CHUNK)
                for (f0, ra, rb, dst) in ((0, qra, qrb, qp),
                                          (256, kra, krb, kp)):
                    pa = pp.tile([128, CHUNK], f32, tag="pa", name="pa")
                    pb = pp.tile([128, CHUNK], f32, tag="pb", name="pb")
                    for e in range(ESUB):
                        kw = dict(start=(e == 0), stop=(e == ESUB - 1))
                        xs = xT_sb[:, e, csl]
                        nc.tensor.matmul(pa[:], wqk_sb[:, e, f0:f0 + 128], xs, **kw)
                        nc.tensor.matmul(pb[:], wqk_sb[:, e, f0 + 128:f0 + 256], xs, **kw)
                    # rope: PSUM-reading mults on DVE, SBUF-only add/sub on Pool
                    t1 = rsc.tile([128, CHUNK], f32, tag="t1", name="t1")
                    t2 = rsc.tile([128, CHUNK], f32, tag="t2", name="t2")
                    nc.vector.tensor_tensor(t1[:], pa[:], cs_sb[:, csl], Alu.mult)
                    nc.vector.tensor_tensor(t2[:], pb[:], sn_sb[:, csl], Alu.mult)
                    nc.gpsimd.tensor_tensor(ra[:, csl], t1[:], t2[:], Alu.subtract)
                    t3 = rsc.tile([128, CHUNK], f32, tag="t1", name="t3")
                    t4 = rsc.tile([128, CHUNK], f32, tag="t2", name="t4")
                    nc.vector.tensor_tensor(t3[:], pa[:], sn_sb[:, csl], Alu.mult)
                    nc.vector.tensor_tensor(t4[:], pb[:], cs_sb[:, csl], Alu.mult)
                    nc.gpsimd.tensor_tensor(rb[:, csl], t3[:], t4[:], Alu.add)
                    for p in range(2):
                        h0, h1 = 2 * p, 2 * p + 1
                        nc.sync.dma_start(dst[0:32, p, csl],
                                          ra[32 * h0:32 * h0 + 32, csl])
                        nc.sync.dma_start(dst[32:64, p, csl],
                                          rb[32 * h0:32 * h0 + 32, csl])
                        nc.sync.dma_start(dst[64:96, p, csl],
                                          ra[32 * h1:32 * h1 + 32, csl])
                        nc.sync.dma_start(dst[96:128, p, csl],
                                          rb[32 * h1:32 * h1 + 32, csl])

            def emit_attn(bb):
                # scores+exp run one j-tile ahead of PV so exp(jt) hides
                # under PV(jt-1)+scores(jt+1) on the PE
                i0 = bb * BLK
                njt = 4 * bb + 4

                def s_and_e(p, jt):
                    r = jt - 4 * bb
                    off = 128 * max(r, 0)
                    ps_s = pp.tile([128, 2, BLK], f32, tag="s",
                                   name="ps_s", bufs=2)
                    for a in range(2):
                        nc.tensor.matmul(
                            ps_s[:, a, off:],
                            kp[64 * a:64 * a + 64, p,
                               128 * jt:128 * jt + 128],
                            qp[64 * a:64 * a + 64, p,
                               i0 + off:i0 + BLK],
                            start=True, stop=True)
                    pt = ptp.tile([128, 2, BLK], f16, tag="pt", name="pt")
                    nc.scalar.activation(pt[:, :, off:], ps_s[:, :, off:],
                                         Act.Exp, scale=scale)
                    if r >= 0:
                        # triangle mask on the otherwise-idle Pool engine
                        nc.gpsimd.tensor_tensor(
                            pt[:, :, off:off + 128],
                            pt[:, :, off:off + 128],
                            tri_sb[:, None, :].to_broadcast((128, 2, 128)),
                            Alu.mult)
                    return pt, off

                ctxs = {}
                for p in range(2):
                    ctx = [pp.tile([65, BLK], f32, tag=f"ctx{a}",
                                   name=f"ctx{a}") for a in range(2)]
                    ctxs[p] = ctx

                    def pv(jt, pt, off):
                        # per-element has_written handles the ragged causal
                        # ranges; the 2KB-granular group check cannot
                        for a in range(2):
                            nc.tensor.matmul(
                                ctx[a][:, off:],
                                v_sb[:, jt, 65 * (2 * p + a):
                                     65 * (2 * p + a) + 65],
                                pt[:, a, off:],
                                start=(jt == 0), stop=(jt == njt - 1),
                                skip_group_check=True)

                    prev = None
                    for jt in range(njt):
                        cur = s_and_e(p, jt)
                        if prev is not None:
                            pv(jt - 1, *prev)
                        prev = cur
                    pv(njt - 1, *prev)
                    # stash unnormalized ctx + Z rows (DVE)
                    for a in range(2):
                        nc.vector.tensor_copy(
                            ctxu_sb[64 * a:64 * a + 64, p, i0:i0 + BLK],
                            ctx[a][0:64, :])
                        zst = smp.tile([1, BLK], f32, tag="zst", name="zst",
                                       bufs=4)
                        nc.vector.tensor_copy(zst[:], ctx[a][64:65, :])
                        nc.sync.dma_start(
                            zall32[2 * p + a:2 * p + a + 1, i0:i0 + BLK],
                            zst[:])
                return ctxs

            def emit_norm(bb, ctxs):
                # 1/Z broadcast via selector matmul; the stash copy is fused
                # with normalization: ctxu = ctx * (1/Z) in one DVE op each
                qsl = slice(bb * BLK, (bb + 1) * BLK)
                rz = smp.tile([4, BLK], f32, tag="rz", name="rz")
                nc.vector.reciprocal_approx_fast(rz[:], zall32[:, qsl])
                nc.gpsimd.tensor_copy(zall16[:, qsl], rz[:])
                for p in range(2):
                    zt = pp.tile([128, 2, BLK], f32, tag="s", name="zt",
                                 bufs=2)
                    nc.tensor.matmul(zt[:, 0, :],
                                     sel_sb[:, 128 * p:128 * p + 128],
                                     zall16[:, qsl], start=True, stop=True)
                    nc.vector.tensor_tensor(ctxu_sb[:, p, qsl],
                                            ctxu_sb[:, p, qsl],
                                            zt[:, 0, :], Alu.mult)

            def emit_outproj(bb, late):
                # po on pa/pb tags only (ring 2, free after the projections);
                # emitted LAST so the Tile list-scheduler backfills these
                # matmuls into the exp-gated PE gaps of attention blocks 2-3,
                # which keeps the PE continuously busy and in max p-state
                for k in range(4):
                    st = 4 * bb + k
                    ssl = slice(st * 128, (st + 1) * 128)
                    for n in range(2):
                        po = pp.tile([128, CHUNK], f32,
                                     tag=("pa" if n == 0 else "pb"),
                                     name="po")[:]
                        nsl = slice(n * 512, (n + 1) * 512)
                        nc.tensor.matmul(po, ctxu_sb[:, 0, ssl],
                                         wout_sb[:, 0, nsl],
                                         start=True, stop=False)
                        nc.tensor.matmul(po, ctxu_sb[:, 1, ssl],
                                         wout_sb[:, 1, nsl],
                                         start=False, stop=True)
                        ot = otp.tile([128, 512], f16, tag="ot", name="ot")
                        if late:
                            nc.vector.tensor_copy(ot[:, 0:256], po[:, 0:256])
                            nc.scalar.copy(ot[:, 256:512], po[:, 256:512])
                        else:
                            nc.vector.tensor_copy(ot[:], po[:])
                        nc.sync.dma_start(out_d[ssl, nsl], ot[:])

            # ---- main stream ----
            emit_vproj(0)
            emit_qkproj(0)
            emit_vproj(1)
            emit_qkproj(1)
            for bb in range(NBLK):
                ctxs = emit_attn(bb)
                emit_norm(bb, ctxs)
                if bb + 2 < NCHUNK:
                    emit_vproj(bb + 2)
                    emit_qkproj(bb + 2)
            for bb in range(NBLK):
                emit_outproj(bb, late=(bb >= 2))

    nc.compile()
    return nc


def _host_inputs(x, W_qkv, W_out):
    """Build the 8 per-core input maps."""
    x = np.asarray(x, dtype=np.float32)
    W_qkv = np.asarray(W_qkv, dtype=np.float32)
    W_out = np.asarray(W_out, dtype=np.float32)

    pos = np.arange(S)
    freqs = 1.0 / 10000.0 ** (np.arange(0, HEAD_DIM, 2) / HEAD_DIM)
    ang = pos[:, None] * freqs[None, :]            # (S, 32)
    cs32 = np.cos(ang).T.astype(np.float32)        # (32, S)
    sn32 = np.sin(ang).T.astype(np.float32)
    cs = np.tile(cs32, (4, 1))                     # (128, S)
    sn = np.tile(sn32, (4, 1))
    tri = (np.arange(128)[:, None] <= np.arange(128)[None, :]).astype(np.float16)
    # selector for Z broadcast: sel[k, 128p+m] = 1 where k == 2p + m//64
    sel = np.zeros((4, 256), np.float16)
    for p in range(2):
        for m in range(128):
            sel[2 * p + m // 64, 128 * p + m] = 1.0

    in_maps = []
    for b in range(B):
        xT = np.ascontiguousarray(x[b].T.astype(np.float16))
        for g in range(NG):
            heads = np.arange(HG * g, HG * g + HG)
            qa = np.concatenate([0 * NUM_HEADS * HEAD_DIM + h * HEAD_DIM
                                 + np.arange(0, HEAD_DIM, 2) for h in heads])
            qb = qa + 1
            ka = qa + NUM_HEADS * HEAD_DIM
            kb = ka + 1
            wqk = np.ascontiguousarray(
                W_qkv[:, np.concatenate([qa, qb, ka, kb])].astype(np.float16))
            vcols = np.concatenate([2 * NUM_HEADS * HEAD_DIM + h * HEAD_DIM
                                    + np.arange(HEAD_DIM) for h in heads])
            wv = np.ascontiguousarray(W_qkv[:, vcols].astype(np.float16))
            wout = np.ascontiguousarray(
                W_out[HG * g * HEAD_DIM:HG * (g + 1) * HEAD_DIM].astype(np.float16))
            in_maps.append({"xT": xT, "wqk": wqk, "wv": wv, "wout": wout,
                            "cs": cs, "sn": sn, "tri": tri, "sel": sel})
    return in_maps


def get_program():
    if "nc" not in _CACHE:
        _CACHE["nc"] = _build_program()
    return _CACHE["nc"]


def run(x, W_qkv, W_out, trace=False, tmpdir=None):
    from concourse import bass_utils
    nc = get_program()
    in_maps = _host_inputs(x, W_qkv, W_out)
    res = bass_utils.run_bass_kernel_spmd(
        nc, in_maps, core_ids=list(range(N_CORES)), trace=trace, tmpdir=tmpdir)
    out = np.zeros((B, S, E), np.float32)
    for b in range(B):
        for g in range(NG):
            out[b] += res.results[b * NG + g]["out"].astype(np.float32)
    return out, res


def kernel(x, W_qkv, W_out):
    out, _ = run(x, W_qkv, W_out)
    return out


# revision 15
# speedup vs baseline: 1.1627x; 1.0208x over previous
"""Causal self-attention (RoPE) Trainium2 kernel, 8-way sharded.

Sharding: core = (batch b in 0..1) x (head group g in 0..3, 4 heads each).
Each core computes its batch's attention for its 4 heads plus the partial
output projection; the host sums the 4 partials per batch.

Layout strategy (per core):
- host passes xT = x[b].T (fp16) so the embed dim lands on SBUF partitions.
- W_qkv columns are permuted so q^T/k^T emerge from the projection matmul
  already transposed, with RoPE even/odd dim pairs de-interleaved into
  x1/x2 partition blocks (scores are invariant to a head-dim permutation).
- all matmul operands are fp16 (1 cycle/row on PE vs 4 for fp32); PSUM
  accumulation stays fp32. End-to-end error ~5e-4.
- scores are computed transposed (sT[j,i]); softmax needs no max pass
  (|scores| < ~4) and the denominator is obtained by appending a ones
  column to V (M=65 PV matmuls). Normalization happens per i-block via a
  selector-matmul broadcast of 1/Z (DVE reciprocal_approx_fast).
- causal masking: only j<=i column ranges are computed; the single
  diagonal 128x128 block per j-tile gets a triangle multiply (DVE, fp16).

Scheduling strategy (the perf-critical part):
- Everything is one interleaved stream: per 512-seq chunk c we emit
  qk-projection+rope for chunk c, then the output projection of i-block
  c-1, then attention+normalization of i-block c. ACT does only softmax
  exp (~68us); all other elementwise work is spread over DVE and Pool so
  exp overlaps projection and PE (the real bottleneck, ~115us of moving
  rows) never waits long enough to drop out of its 2.4GHz p-state.
- PSUM is exactly 8 banks: pa+pb (proj, 1 each) + scores ring (2x2) +
  2 ctx accumulators; out-proj tiles reuse pa/pb slots (same shape) and
  the Z-broadcast tile reuses the scores ring.
"""
import sys

sys.path.insert(0, "/opt/trn_rl_repo")

import numpy as np

NUM_HEADS = 16
HEAD_DIM = 64
B, S, E = 2, 2048, 1024
HG = 4                      # heads per core
NG = NUM_HEADS // HG        # head groups
N_CORES = B * NG
F_QK = 2 * HG * HEAD_DIM    # 512 projected q+k rows per core
F_V = HG * HEAD_DIM         # 256 v cols per core
ESUB = E // 128             # 8 K-subtiles over embed dim
NCHUNK = 4                  # 512-col seq chunks (projection)
CHUNK = S // NCHUNK         # 512
NST = S // 128              # 16 seq tiles of 128
BLK = 512                   # attention i-block width
NBLK = S // BLK             # 4

_CACHE = {}


def _build_program():
    import concourse.bass as bass
    import concourse.mybir as mybir
    import concourse.tile as tile
    from concourse import bacc

    f32 = mybir.dt.float32
    f16 = mybir.dt.float16
    Alu = mybir.AluOpType
    Act = mybir.ActivationFunctionType

    nc = bacc.Bacc("TRN2", target_bir_lowering=False, debug=False,
                   num_devices=N_CORES)

    xT_d = nc.dram_tensor("xT", (E, S), f16, kind="ExternalInput").ap()
    wqk_d = nc.dram_tensor("wqk", (E, F_QK), f16, kind="ExternalInput").ap()
    wv_d = nc.dram_tensor("wv", (E, F_V), f16, kind="ExternalInput").ap()
    wout_d = nc.dram_tensor("wout", (F_V, E), f16, kind="ExternalInput").ap()
    cs_d = nc.dram_tensor("cs", (128, S), f32, kind="ExternalInput").ap()
    sn_d = nc.dram_tensor("sn", (128, S), f32, kind="ExternalInput").ap()
    tri_d = nc.dram_tensor("tri", (128, 128), f16, kind="ExternalInput").ap()
    sel_d = nc.dram_tensor("sel", (4, 256), f16, kind="ExternalInput").ap()
    out_d = nc.dram_tensor("out", (S, E), f16, kind="ExternalOutput").ap()

    scale = 1.0 / float(np.sqrt(HEAD_DIM))

    with tile.TileContext(nc) as tc:
        with tc.tile_pool(name="wk", bufs=1) as wp, \
             tc.tile_pool(name="rsc", bufs=3) as rsc, \
             tc.tile_pool(name="pt", bufs=4) as ptp, \
             tc.tile_pool(name="sm", bufs=2) as smp, \
             tc.tile_pool(name="ot", bufs=4) as otp, \
             tc.tile_pool(name="pp", bufs=1, space="PSUM") as pp:
            # ---- persistent SBUF tensors ----
            xT_sb = wp.tile([128, ESUB, S], f16)
            wv_sb = wp.tile([128, ESUB, F_V], f16)
            wqk_sb = wp.tile([128, ESUB, F_QK], f16)
            wout_sb = wp.tile([128, 2, E], f16)
            cs_sb = wp.tile([128, S], f32)
            sn_sb = wp.tile([128, S], f32)
            tri_sb = wp.tile([128, 128], f16)
            sel_sb = wp.tile([4, 256], f16)
            v_sb = wp.tile([128, NST, HG * 65], f16)
            ctxu_sb = wp.tile([128, 2, S], f16)
            zall32 = wp.tile([4, S], f32)
            zall16 = wp.tile([4, S], f16)
            qra = wp.tile([128, S], f16)
            qrb = wp.tile([128, S], f16)
            kra = wp.tile([128, S], f16)
            krb = wp.tile([128, S], f16)
            qp = wp.tile([128, 2, S], f16)
            kp = wp.tile([128, 2, S], f16)

            # ---- input DMAs, ordered so vproj can start ASAP ----
            xT_r = xT_d.rearrange("(o p) s -> p o s", p=128)
            nc.sync.dma_start(wv_sb[:], wv_d.rearrange("(o p) f -> p o f", p=128))
            nc.sync.dma_start(xT_sb[:, :, 0:256], xT_r[:, :, 0:256])
            nc.sync.dma_start(xT_sb[:, :, 256:512], xT_r[:, :, 256:512])
            nc.sync.dma_start(wqk_sb[:], wqk_d.rearrange("(o p) f -> p o f", p=128))
            nc.sync.dma_start(cs_sb[:], cs_d[:])
            nc.sync.dma_start(sn_sb[:], sn_d[:])
            for c in range(1, NCHUNK):
                csl = slice(c * CHUNK, (c + 1) * CHUNK)
                nc.sync.dma_start(xT_sb[:, :, csl], xT_r[:, :, csl])
            nc.sync.dma_start(tri_sb[:], tri_d[:])
            nc.sync.dma_start(sel_sb[:], sel_d[:])
            nc.sync.dma_start(wout_sb[:],
                              wout_d.rearrange("(o p) e -> p o e", p=128))

            # ones columns of v (only the 65th col of each head slot)
            nc.gpsimd.memset(
                v_sb[:].rearrange("p st (h w) -> p st h w", h=HG)[:, :, :, 64:65],
                1.0)

            # ---- emission helpers ----
            def emit_vproj(c):
                for st in range(4 * c, 4 * c + 4):
                    ssl = slice(st * 128, (st + 1) * 128)
                    pv = pp.tile([128, CHUNK], f32, tag="pa", name="pv")
                    for e in range(ESUB):
                        nc.tensor.matmul(pv[:, 0:F_V], xT_sb[:, e, ssl],
                                         wv_sb[:, e, :],
                                         start=(e == 0), stop=(e == ESUB - 1))
                    nc.vector.tensor_copy(
                        v_sb[:, st, :].rearrange("p (h w) -> p h w", h=HG)[:, :, 0:64],
                        pv[:, 0:F_V].rearrange("p (h w) -> p h w", h=HG))

            def emit_qkproj(c):
                csl = slice(c * CHUNK, (c + 1) * CHUNK)
                for (f0, ra, rb, dst) in ((0, qra, qrb, qp),
                                          (256, kra, krb, kp)):
                    pa = pp.tile([128, CHUNK], f32, tag="pa", name="pa")
                    pb = pp.tile([128, CHUNK], f32, tag="pb", name="pb")
                    for e in range(ESUB):
                        kw = dict(start=(e == 0), stop=(e == ESUB - 1))
                        xs = xT_sb[:, e, csl]
                        nc.tensor.matmul(pa[:], wqk_sb[:, e, f0:f0 + 128], xs, **kw)
                        nc.tensor.matmul(pb[:], wqk_sb[:, e, f0 + 128:f0 + 256], xs, **kw)
                    # rope: PSUM-reading mults on DVE, SBUF-only add/sub on Pool
                    t1 = rsc.tile([128, CHUNK], f32, tag="t1", name="t1")
                    t2 = rsc.tile([128, CHUNK], f32, tag="t2", name="t2")
                    nc.vector.tensor_tensor(t1[:], pa[:], cs_sb[:, csl], Alu.mult)
                    nc.vector.tensor_tensor(t2[:], pb[:], sn_sb[:, csl], Alu.mult)
                    nc.gpsimd.tensor_tensor(ra[:, csl], t1[:], t2[:], Alu.subtract)
                    t3 = rsc.tile([128, CHUNK], f32, tag="t1", name="t3")
                    t4 = rsc.tile([128, CHUNK], f32, tag="t2", name="t4")
                    nc.vector.tensor_tensor(t3[:], pa[:], sn_sb[:, csl], Alu.mult)
                    nc.vector.tensor_tensor(t4[:], pb[:], cs_sb[:, csl], Alu.mult)
                    nc.gpsimd.tensor_tensor(rb[:, csl], t3[:], t4[:], Alu.add)
                    for p in range(2):
                        h0, h1 = 2 * p, 2 * p + 1
                        nc.sync.dma_start(dst[0:32, p, csl],
                                          ra[32 * h0:32 * h0 + 32, csl])
                        nc.sync.dma_start(dst[32:64, p, csl],
                                          rb[32 * h0:32 * h0 + 32, csl])
                        nc.sync.dma_start(dst[64:96, p, csl],
                                          ra[32 * h1:32 * h1 + 32, csl])
                        nc.sync.dma_start(dst[96:128, p, csl],
                                          rb[32 * h1:32 * h1 + 32, csl])

            def emit_attn(bb):
                # scores+exp run one j-tile ahead of PV so exp(jt) hides
                # under PV(jt-1)+scores(jt+1) on the PE
                i0 = bb * BLK
                njt = 4 * bb + 4

                def s_and_e(p, jt):
                    r = jt - 4 * bb
                    off = 128 * max(r, 0)
                    ps_s = pp.tile([128, 2, BLK], f32, tag="s",
                                   name="ps_s", bufs=2)
                    for a in range(2):
                        nc.tensor.matmul(
                            ps_s[:, a, off:],
                            kp[64 * a:64 * a + 64, p,
                               128 * jt:128 * jt + 128],
                            qp[64 * a:64 * a + 64, p,
                               i0 + off:i0 + BLK],
                            start=True, stop=True)
                    pt = ptp.tile([128, 2, BLK], f16, tag="pt", name="pt")
                    nc.scalar.activation(pt[:, :, off:], ps_s[:, :, off:],
                                         Act.Exp, scale=scale)
                    if r >= 0:
                        # triangle mask on the otherwise-idle Pool engine
                        nc.gpsimd.tensor_tensor(
                            pt[:, :, off:off + 128],
                            pt[:, :, off:off + 128],
                            tri_sb[:, None, :].to_broadcast((128, 2, 128)),
                            Alu.mult)
                    return pt, off

                ctxs = {}
                for p in range(2):
                    ctx = [pp.tile([65, BLK], f32, tag=f"ctx{a}",
                                   name=f"ctx{a}") for a in range(2)]
                    ctxs[p] = ctx

                    def pv(jt, pt, off):
                        # per-element has_written handles the ragged causal
                        # ranges; the 2KB-granular group check cannot
                        for a in range(2):
                            nc.tensor.matmul(
                                ctx[a][:, off:],
                                v_sb[:, jt, 65 * (2 * p + a):
                                     65 * (2 * p + a) + 65],
                                pt[:, a, off:],
                                start=(jt == 0), stop=(jt == njt - 1),
                                skip_group_check=True)

                    prev = None
                    for jt in range(njt):
                        cur = s_and_e(p, jt)
                        if prev is not None:
                            pv(jt - 1, *prev)
                        prev = cur
                    pv(njt - 1, *prev)
                    # stash unnormalized ctx + Z rows (DVE)
                    for a in range(2):
                        nc.vector.tensor_copy(
                            ctxu_sb[64 * a:64 * a + 64, p, i0:i0 + BLK],
                            ctx[a][0:64, :])
                        zst = smp.tile([1, BLK], f32, tag="zst", name="zst",
                                       bufs=4)
                        nc.vector.tensor_copy(zst[:], ctx[a][64:65, :])
                        nc.sync.dma_start(
                            zall32[2 * p + a:2 * p + a + 1, i0:i0 + BLK],
                            zst[:])
                return ctxs

            def emit_norm(bb, ctxs):
                # 1/Z broadcast via selector matmul; the stash copy is fused
                # with normalization: ctxu = ctx * (1/Z) in one DVE op each
                qsl = slice(bb * BLK, (bb + 1) * BLK)
                rz = smp.tile([4, BLK], f32, tag="rz", name="rz")
                nc.vector.reciprocal_approx_fast(rz[:], zall32[:, qsl])
                nc.gpsimd.tensor_copy(zall16[:, qsl], rz[:])
                for p in range(2):
                    zt = pp.tile([128, 2, BLK], f32, tag="s", name="zt",
                                 bufs=2)
                    nc.tensor.matmul(zt[:, 0, :],
                                     sel_sb[:, 128 * p:128 * p + 128],
                                     zall16[:, qsl], start=True, stop=True)
                    nc.vector.tensor_tensor(ctxu_sb[:, p, qsl],
                                            ctxu_sb[:, p, qsl],
                                            zt[:, 0, :], Alu.mult)

            def emit_outproj(bb, late):
                # po on pa/pb tags only (ring 2, free after the projections);
                # emitted LAST so the Tile list-scheduler backfills these
                # matmuls into the exp-gated PE gaps of attention blocks 2-3,
                # which keeps the PE continuously busy and in max p-state
                for k in range(4):
                    st = 4 * bb + k
                    ssl = slice(st * 128, (st + 1) * 128)
                    for n in range(2):
                        po = pp.tile([128, CHUNK], f32,
                                     tag=("pa" if n == 0 else "pb"),
                                     name="po")[:]
                        nsl = slice(n * 512, (n + 1) * 512)
                        nc.tensor.matmul(po, ctxu_sb[:, 0, ssl],
                                         wout_sb[:, 0, nsl],
                                         start=True, stop=False)
                        nc.tensor.matmul(po, ctxu_sb[:, 1, ssl],
                                         wout_sb[:, 1, nsl],
                                         start=False, stop=True)
                        ot = otp.tile([128, 512], f16, tag="ot", name="ot")
                        if late:
                            nc.vector.tensor_copy(ot[:, 0:256], po[:, 0:256])
                            nc.scalar.copy(ot[:, 256:512], po[:, 256:512])
                        else:
                            nc.vector.tensor_copy(ot[:], po[:])
                        nc.sync.dma_start(out_d[ssl, nsl], ot[:])

            # ---- main stream ----
            # proj for chunk c+1 is emitted AFTER attention block c so the
            # list scheduler treats attention as higher priority and uses
            # the proj matmuls to backfill exp-gated PE gaps, instead of
            # front-running them (which starves ACT at block boundaries)
            emit_vproj(0)
            emit_qkproj(0)
            emit_vproj(1)
            emit_qkproj(1)
            for bb in range(NBLK):
                ctxs = emit_attn(bb)
                emit_norm(bb, ctxs)
                if bb + 1 < NBLK and bb + 1 >= 1:
                    c = bb + 1
                    if c >= 2:
                        emit_vproj(c)
                        emit_qkproj(c)
            for bb in range(NBLK):
                emit_outproj(bb, late=(bb >= 2))

    nc.compile()
    return nc


def _host_inputs(x, W_qkv, W_out):
    """Build the 8 per-core input maps."""
    x = np.asarray(x, dtype=np.float32)
    W_qkv = np.asarray(W_qkv, dtype=np.float32)
    W_out = np.asarray(W_out, dtype=np.float32)

    pos = np.arange(S)
    freqs = 1.0 / 10000.0 ** (np.arange(0, HEAD_DIM, 2) / HEAD_DIM)
    ang = pos[:, None] * freqs[None, :]            # (S, 32)
    cs32 = np.cos(ang).T.astype(np.float32)        # (32, S)
    sn32 = np.sin(ang).T.astype(np.float32)
    cs = np.tile(cs32, (4, 1))                     # (128, S)
    sn = np.tile(sn32, (4, 1))
    tri = (np.arange(128)[:, None] <= np.arange(128)[None, :]).astype(np.float16)
    # selector for Z broadcast: sel[k, 128p+m] = 1 where k == 2p + m//64
    sel = np.zeros((4, 256), np.float16)
    for p in range(2):
        for m in range(128):
            sel[2 * p + m // 64, 128 * p + m] = 1.0

    in_maps = []
    for b in range(B):
        xT = np.ascontiguousarray(x[b].T.astype(np.float16))
        for g in range(NG):
            heads = np.arange(HG * g, HG * g + HG)
            qa = np.concatenate([0 * NUM_HEADS * HEAD_DIM + h * HEAD_DIM
                                 + np.arange(0, HEAD_DIM, 2) for h in heads])
            qb = qa + 1
            ka = qa + NUM_HEADS * HEAD_DIM
            kb = ka + 1
            wqk = np.ascontiguousarray(
                W_qkv[:, np.concatenate([qa, qb, ka, kb])].astype(np.float16))
            vcols = np.concatenate([2 * NUM_HEADS * HEAD_DIM + h * HEAD_DIM
                                    + np.arange(HEAD_DIM) for h in heads])
            wv = np.ascontiguousarray(W_qkv[:, vcols].astype(np.float16))
            wout = np.ascontiguousarray(
                W_out[HG * g * HEAD_DIM:HG * (g + 1) * HEAD_DIM].astype(np.float16))
            in_maps.append({"xT": xT, "wqk": wqk, "wv": wv, "wout": wout,
                            "cs": cs, "sn": sn, "tri": tri, "sel": sel})
    return in_maps


def get_program():
    if "nc" not in _CACHE:
        _CACHE["nc"] = _build_program()
    return _CACHE["nc"]


def run(x, W_qkv, W_out, trace=False, tmpdir=None):
    from concourse import bass_utils
    nc = get_program()
    in_maps = _host_inputs(x, W_qkv, W_out)
    res = bass_utils.run_bass_kernel_spmd(
        nc, in_maps, core_ids=list(range(N_CORES)), trace=trace, tmpdir=tmpdir)
    out = np.zeros((B, S, E), np.float32)
    for b in range(B):
        for g in range(NG):
            out[b] += res.results[b * NG + g]["out"].astype(np.float32)
    return out, res


def kernel(x, W_qkv, W_out):
    out, _ = run(x, W_qkv, W_out)
    return out
